# revision 1
# baseline (speedup 1.0000x reference)
"""DeepSpeedMLP (pre-LN fp32 path) on 8 Trainium2 NeuronCores.

Sharding: data-parallel over tokens (8192 tokens -> 1024/core); every core
streams the full inter_w/output_w from HBM exactly once while holding the
transposed LN activations and the current I-block of gelu activations
SBUF-resident.  Matmuls run as float32r (fp22 mantissa, 1 PE pass) giving
~1e-4 relative error at bf16-class throughput.

Per-core pipeline:
  stage 1: res = input+residual+bias; LN; 128x128 PE transposes -> lnT
           (gamma/beta fused into the PSUM->SBUF copy); out is seeded
           with res (the final residual add).
  stage 2: for each 2048-wide I-block:
           GEMM1  psum[t,i] += lnT_k.T @ W1[k, iblock]   (lnT stationary)
           evict: psum -> f32 stage -> PE transpose -> gelu(x+b1) -> inter
           GEMM2  psum[t,h] += inter_i.T @ W2[iblock, h] (inter stationary)
           out[t,h] += psum  (DMA load/add/store; output_b added via a
           K=1 ones-matmul into the last block's psum accumulation)
"""
import sys
if '/opt/trn_rl_repo' not in sys.path:
    sys.path.insert(0, '/opt/trn_rl_repo')

import numpy as np
import concourse.bass as bass
import concourse.mybir as mybir
import concourse.tile as tile
from concourse import bacc
from concourse.bass_utils import run_bass_kernel_spmd

dt = mybir.dt
AF = mybir.ActivationFunctionType
ALU = mybir.AluOpType

N_CORES = 8
B, S, HIDDEN, INTER = 4, 2048, 4096, 16384
TOK = B * S
T = TOK // N_CORES       # tokens per core
IBLK = 2048              # I-block width
EPS = 1e-5


def _build_nc(H, I, T, IBLK):
    KS = H // 128     # H k-slabs
    TT = T // 128     # token tiles
    NB = I // IBLK    # I blocks
    IC = IBLK // 512  # 512-wide i-chunks per block (GEMM1 psum N)
    IS = IBLK // 128  # 128-row i-slabs per block (GEMM2 lhsT)
    HC = H // 512     # 512-wide h-chunks (GEMM2 psum N)
    SW = min(H, 2048)  # stage-1 strip width
    NS = H // SW

    nc = bacc.Bacc(None, target_bir_lowering=False)
    P = nc.declare_dram_parameter
    x_d = P("x", [T, H], dt.float32, isOutput=False)
    r_d = P("r", [T, H], dt.float32, isOutput=False)
    g_d = P("gamma_t", [128, KS], dt.float32, isOutput=False)
    be_d = P("beta_t", [128, KS], dt.float32, isOutput=False)
    w1_d = P("w1", [H, I], dt.float32r, isOutput=False)
    b1_d = P("b1_t", [128, I // 128], dt.float32, isOutput=False)
    w2_d = P("w2", [I, H], dt.float32r, isOutput=False)
    b2_d = P("b2", [1, H], dt.float32r, isOutput=False)
    ones_d = P("ones", [1, 128], dt.float32r, isOutput=False)
    id_d = P("ident", [128, 128], dt.float32, isOutput=False)
    o_d = P("out", [T, H], dt.float32, isOutput=True)

    with tile.TileContext(nc) as tc:
        with (
            tc.tile_pool(name="const", bufs=1) as constp,
            tc.tile_pool(name="lnT", bufs=1) as lnTp,
            tc.tile_pool(name="psum", bufs=8, space="PSUM") as psum,
        ):
            ident = constp.tile([128, 128], dt.float32)
            nc.sync.dma_start(out=ident[:], in_=id_d[:])
            g_sb = constp.tile([128, KS], dt.float32)
            nc.sync.dma_start(out=g_sb[:], in_=g_d[:])
            be_sb = constp.tile([128, KS], dt.float32)
            nc.sync.dma_start(out=be_sb[:], in_=be_d[:])
            b1_sb = constp.tile([128, I // 128], dt.float32)
            nc.sync.dma_start(out=b1_sb[:], in_=b1_d[:])
            ones = constp.tile([1, 128], dt.float32r)
            nc.sync.dma_start(out=ones[:], in_=ones_d[:])

            lnT = lnTp.tile([128, KS, T], dt.float32r)

            # ---- Stage 1: residual add + LN + transpose ----
            with (
                tc.tile_pool(name="s1in", bufs=2) as inp,
                tc.tile_pool(name="s1res", bufs=1) as resp,
                tc.tile_pool(name="s1st", bufs=2) as stp,
            ):
                for t in range(TT):
                    tr = slice(t * 128, (t + 1) * 128)
                    res = resp.tile([128, H], dt.float32, name="res")
                    for s in range(NS):
                        cs = slice(s * SW, (s + 1) * SW)
                        xs = inp.tile([128, SW], dt.float32, name="xt")
                        rs = inp.tile([128, SW], dt.float32, name="rt")
                        nc.sync.dma_start(out=xs[:], in_=x_d[tr, cs])
                        nc.sync.dma_start(out=rs[:], in_=r_d[tr, cs])
                        nc.vector.tensor_add(res[:, cs], xs[:], rs[:])
                    nc.sync.dma_start(out=o_d[tr, :], in_=res[:])

                    s1 = stp.tile([128, 1], dt.float32, name="s1")
                    nc.vector.tensor_reduce(s1[:], res[:], mybir.AxisListType.X, ALU.add)
                    s2 = stp.tile([128, 1], dt.float32, name="s2")
                    for s in range(NS):
                        cs = slice(s * SW, (s + 1) * SW)
                        junk = inp.tile([128, SW], dt.float32, name="xt")
                        s2p = stp.tile([128, 1], dt.float32, name="s2p")
                        nc.scalar.activation(junk[:], res[:, cs], AF.Square,
                                             accum_out=s2p[:])
                        if s == 0:
                            nc.vector.tensor_copy(s2[:], s2p[:])
                        else:
                            nc.vector.tensor_add(s2[:], s2[:], s2p[:])
                    mu = stp.tile([128, 1], dt.float32, name="mu")
                    nc.vector.tensor_scalar_mul(mu[:], s1[:], 1.0 / H)
                    mu2 = stp.tile([128, 1], dt.float32, name="mu2")
                    nc.vector.tensor_mul(mu2[:], mu[:], mu[:])
                    var = stp.tile([128, 1], dt.float32, name="var")
                    nc.vector.tensor_scalar(var[:], s2[:], 1.0 / H, float(EPS),
                                            ALU.mult, ALU.add)
                    nc.vector.tensor_sub(var[:], var[:], mu2[:])
                    sd = stp.tile([128, 1], dt.float32, name="sd")
                    nc.scalar.activation(sd[:], var[:], AF.Sqrt)
                    rstd = stp.tile([128, 1], dt.float32, name="rstd")
                    nc.vector.reciprocal(rstd[:], sd[:])
                    nmr = stp.tile([128, 1], dt.float32, name="nmr")
                    nc.vector.tensor_mul(nmr[:], mu[:], rstd[:])
                    nc.vector.tensor_scalar_mul(nmr[:], nmr[:], -1.0)

                    for s in range(NS):
                        cs = slice(s * SW, (s + 1) * SW)
                        lnp = inp.tile([128, SW], dt.float32, name="rt")
                        nc.scalar.activation(lnp[:], res[:, cs], AF.Identity,
                                             bias=nmr[:], scale=rstd[:])
                        for q in range(SW // 512):
                            pt = psum.tile([128, 512], dt.float32, name="ps")
                            for j in range(4):
                                nc.tensor.transpose(
                                    pt[:, j * 128:(j + 1) * 128],
                                    lnp[:, q * 512 + j * 128: q * 512 + (j + 1) * 128],
                                    ident[:])
                            for j in range(4):
                                k = (s * SW + q * 512) // 128 + j
                                nc.vector.tensor_scalar(
                                    lnT[:, k, t * 128:(t + 1) * 128],
                                    pt[:, j * 128:(j + 1) * 128],
                                    g_sb[:, k:k + 1], be_sb[:, k:k + 1],
                                    ALU.mult, ALU.add)

            # ---- Stage 2: per I-block GEMM1 -> gelu -> GEMM2 ----
            with (
                tc.tile_pool(name="interp", bufs=1) as interp,
                tc.tile_pool(name="wt", bufs=3) as wtp,
                tc.tile_pool(name="stage", bufs=3) as stgp,
                tc.tile_pool(name="b2sl", bufs=1) as b2p,
            ):
                inter = interp.tile([128, IS, T], dt.float32r)
                for b in range(NB):
                    for ic in range(IC):
                        pA = [psum.tile([128, 512], dt.float32, name="ps")
                              for _ in range(TT)]
                        for k in range(KS):
                            w1t = wtp.tile([128, 512], dt.float32r, name="wt")
                            nc.sync.dma_start(
                                out=w1t[:],
                                in_=w1_d[k * 128:(k + 1) * 128,
                                         b * IBLK + ic * 512: b * IBLK + (ic + 1) * 512])
                            for t in range(TT):
                                nc.tensor.matmul(
                                    pA[t][:], lnT[:, k, t * 128:(t + 1) * 128], w1t[:],
                                    start=(k == 0), stop=(k == KS - 1))
                        for t in range(TT):
                            s = stgp.tile([128, 512], dt.float32, name="stage")
                            nc.scalar.activation(s[:], pA[t][:], AF.Copy)
                            pT = psum.tile([128, 512], dt.float32, name="ps")
                            for j in range(4):
                                nc.tensor.transpose(
                                    pT[:, j * 128:(j + 1) * 128],
                                    s[:, j * 128:(j + 1) * 128], ident[:])
                            for j in range(4):
                                slab = ic * 4 + j
                                nc.scalar.activation(
                                    inter[:, slab, t * 128:(t + 1) * 128],
                                    pT[:, j * 128:(j + 1) * 128], AF.Gelu_apprx_tanh,
                                    bias=b1_sb[:, b * IS + slab: b * IS + slab + 1])
                    last = (b == NB - 1)
                    for h in range(HC):
                        if last:
                            b2s = b2p.tile([1, 512], dt.float32r, name="b2sl")
                            nc.sync.dma_start(out=b2s[:],
                                              in_=b2_d[:, h * 512:(h + 1) * 512])
                        pB = [psum.tile([128, 512], dt.float32, name="ps")
                              for _ in range(TT)]
                        for i in range(IS):
                            w2t = wtp.tile([128, 512], dt.float32r, name="wt")
                            nc.sync.dma_start(
                                out=w2t[:],
                                in_=w2_d[b * IBLK + i * 128: b * IBLK + (i + 1) * 128,
                                         h * 512:(h + 1) * 512])
                            for t in range(TT):
                                nc.tensor.matmul(
                                    pB[t][:], inter[:, i, t * 128:(t + 1) * 128], w2t[:],
                                    start=(i == 0),
                                    stop=(i == IS - 1) and not last)
                        if last:
                            for t in range(TT):
                                nc.tensor.matmul(pB[t][:], ones[:], b2s[:],
                                                 start=False, stop=True)
                        for t in range(TT):
                            tr = slice(t * 128, (t + 1) * 128)
                            hs = slice(h * 512, (h + 1) * 512)
                            s2t = stgp.tile([128, 512], dt.float32, name="stage")
                            nc.sync.dma_start(out=s2t[:], in_=o_d[tr, hs])
                            nc.vector.tensor_add(s2t[:], s2t[:], pB[t][:])
                            nc.sync.dma_start(out=o_d[tr, hs], in_=s2t[:])
    nc.compile()
    return nc


_NC_CACHE = None


def _get_nc():
    global _NC_CACHE
    if _NC_CACHE is None:
        _NC_CACHE = _build_nc(HIDDEN, INTER, T, IBLK)
    return _NC_CACHE


def kernel(input, residual, residual_norm, bias, attn_nw, attn_nb,
           inter_w, inter_b, output_w, output_b, **kwargs):
    H, I = HIDDEN, INTER
    KS = H // 128
    nc = _get_nc()

    x = np.ascontiguousarray(np.asarray(input, np.float32).reshape(TOK, H))
    r2 = np.asarray(residual, np.float32).reshape(TOK, H) + \
        np.asarray(bias, np.float32)[None, :]
    gamma_t = np.ascontiguousarray(np.asarray(attn_nw, np.float32).reshape(KS, 128).T)
    beta_t = np.ascontiguousarray(np.asarray(attn_nb, np.float32).reshape(KS, 128).T)
    b1_t = np.ascontiguousarray(np.asarray(inter_b, np.float32).reshape(I // 128, 128).T)
    b2 = np.ascontiguousarray(np.asarray(output_b, np.float32)[None, :])
    w1 = np.ascontiguousarray(np.asarray(inter_w, np.float32))
    w2 = np.ascontiguousarray(np.asarray(output_w, np.float32))
    ident = np.eye(128, dtype=np.float32)
    ones = np.ones((1, 128), np.float32)

    maps = []
    for c in range(N_CORES):
        sl = slice(c * T, (c + 1) * T)
        maps.append({
            'x': x[sl], 'r': np.ascontiguousarray(r2[sl]),
            'gamma_t': gamma_t, 'beta_t': beta_t,
            'w1': w1, 'b1_t': b1_t, 'w2': w2, 'b2': b2,
            'ones': ones, 'ident': ident,
        })
    res = run_bass_kernel_spmd(nc, maps, list(range(N_CORES)))
    out = np.concatenate([res.results[c]['out'] for c in range(N_CORES)], 0)
    return out.reshape(B, S, H).astype(np.float32)



# revision 3
# speedup vs baseline: 15.0469x; 15.0469x over previous
"""DeepSpeedMLP (pre-LN fp32 path) on 8 Trainium2 NeuronCores.

Sharding: data-parallel over tokens (8192 tokens -> 1024/core).  Measured
HW: 527 GB/s HBM per core (4.2 TB/s aggregate), 72.6 TF/s fp32r/bf16 per
core, so with bf16 weights (268 MB/core, streamed once per token-half)
weight DMA hides entirely under the ~3.8 ms of matmul.

Dataflow (the key change vs the naive version): GEMM1 is computed
TRANSPOSED -- psum[i,t] = sum_h w1[h,i] * lnT[h,t] -- so its output is
already in the [i-partition, t-free] layout that GEMM2 needs as lhsT.
Gelu+bias evicts PSUM straight into the SBUF `inter` buffer; no
mid-pipeline PE transposes, no staging round-trips.  GEMM2 partials are
accumulated into an SBUF-resident f32 accumulator that stage 1 seeded
with the residual (out = gemm2 + res + output_b), so the DRAM output is
write-once and the kernel never reads its own output buffer.

Per-core pipeline, per token-half (512 tokens; 2 halves so lnT + inter +
accumulator all fit in SBUF):
  stage 1: res = x + (residual+bias); LN stats; normalize; 128x128 PE
           transposes -> lnT[h-slab, t] bf16 (gamma/beta fused into the
           PSUM->SBUF eviction).  res tiles stay resident as the output
           accumulator.
  stage 2: per 2048-wide I-block:
           GEMM1' psum[i,t] += w1[h-slab, i-slab].T @ lnT[h-slab, t]
           evict: gelu(psum + b1) -> inter[i-slab, t] bf16
           GEMM2' psum[t,h] += inter[i-slab, t].T @ w2[i-slab, h]
                  (+ ones.T @ b2 folded into the last block)
           evict: acc[t,h] += psum (DVE)
  final: DMA acc -> out.
"""
import sys
if '/opt/trn_rl_repo' not in sys.path:
    sys.path.insert(0, '/opt/trn_rl_repo')

import numpy as np
import ml_dtypes
import concourse.bass as bass
import concourse.mybir as mybir
import concourse.tile as tile
from concourse import bacc
from concourse.bass_utils import run_bass_kernel_spmd

dt = mybir.dt
AF = mybir.ActivationFunctionType
ALU = mybir.AluOpType

N_CORES = 8
B, S, HIDDEN, INTER = 4, 2048, 4096, 16384
TOK = B * S
T = TOK // N_CORES       # tokens per core
HALF = T // 2            # tokens per half-pass
IBLK = 2048              # I-block width
EPS = 1e-5
WDT = dt.bfloat16        # matmul operand dtype (weights + activations)
WNP = ml_dtypes.bfloat16


def _build_nc(H, I, T, IBLK):
    HALF = T // 2        # tokens per half-pass
    KS = H // 128        # h-slabs (GEMM1' contraction steps)
    NB = I // IBLK       # I blocks
    IS = IBLK // 128     # i-slabs per block
    HC = H // 512        # 512-wide h-chunks (GEMM2' psum N)
    TT = HALF // 128     # token tiles per half
    SW = min(1024, H)    # stage-1 strip width
    NS = H // SW

    nc = bacc.Bacc(None, target_bir_lowering=False)
    P = nc.declare_dram_parameter
    x_d = P("x", [T, H], dt.float32, isOutput=False)
    r_d = P("r", [T, H], dt.float32, isOutput=False)
    g_d = P("gamma_t", [128, KS], dt.float32, isOutput=False)
    be_d = P("beta_t", [128, KS], dt.float32, isOutput=False)
    w1_d = P("w1", [H, I], WDT, isOutput=False)
    b1_d = P("b1_t", [128, I // 128], dt.float32, isOutput=False)
    w2_d = P("w2", [I, H], WDT, isOutput=False)
    b2_d = P("b2", [1, H], WDT, isOutput=False)
    ones_d = P("ones", [1, 128], WDT, isOutput=False)
    id_d = P("ident", [128, 128], dt.float32, isOutput=False)
    o_d = P("out", [T, H], dt.float32, isOutput=True)

    with tile.TileContext(nc) as tc:
        with (
            tc.tile_pool(name="const", bufs=1) as constp,
            tc.tile_pool(name="lnT", bufs=1) as lnTp,
            tc.tile_pool(name="acc", bufs=TT) as accp,
            tc.tile_pool(name="inter", bufs=1) as interp,
            tc.tile_pool(name="s1in", bufs=4) as inp,
            tc.tile_pool(name="s1st", bufs=2) as stp,
            tc.tile_pool(name="wt", bufs=6) as wtp,
            tc.tile_pool(name="psum", bufs=8, space="PSUM") as psum,
        ):
            ident = constp.tile([128, 128], dt.float32)
            nc.sync.dma_start(out=ident[:], in_=id_d[:])
            g_sb = constp.tile([128, KS], dt.float32)
            nc.sync.dma_start(out=g_sb[:], in_=g_d[:])
            be_sb = constp.tile([128, KS], dt.float32)
            nc.sync.dma_start(out=be_sb[:], in_=be_d[:])
            b1_sb = constp.tile([128, I // 128], dt.float32)
            nc.sync.dma_start(out=b1_sb[:], in_=b1_d[:])
            ones = constp.tile([1, 128], WDT)
            nc.sync.dma_start(out=ones[:], in_=ones_d[:])
            b2_sb = constp.tile([1, H], WDT)
            nc.sync.dma_start(out=b2_sb[:], in_=b2_d[:])

            for half in range(2):
                h0 = half * HALF
                lnT = lnTp.tile([128, KS, HALF], WDT, name="lnT")
                accs = []

                # ---- Stage 1: residual add + LN + transpose ----
                for t in range(TT):
                    tr = slice(h0 + t * 128, h0 + (t + 1) * 128)
                    acc = accp.tile([128, H], dt.float32, name="acc")
                    accs.append(acc)
                    for s in range(NS):
                        cs = slice(s * SW, (s + 1) * SW)
                        xs = inp.tile([128, SW], dt.float32, name="xt")
                        rs = inp.tile([128, SW], dt.float32, name="rt")
                        nc.sync.dma_start(out=xs[:], in_=x_d[tr, cs])
                        nc.sync.dma_start(out=rs[:], in_=r_d[tr, cs])
                        nc.vector.tensor_add(acc[:, cs], xs[:], rs[:])

                    s1 = stp.tile([128, 1], dt.float32, name="s1")
                    nc.vector.tensor_reduce(s1[:], acc[:], mybir.AxisListType.X,
                                            ALU.add)
                    s2 = stp.tile([128, 1], dt.float32, name="s2")
                    for s in range(NS):
                        cs = slice(s * SW, (s + 1) * SW)
                        junk = inp.tile([128, SW], dt.float32, name="xt")
                        s2p = stp.tile([128, 1], dt.float32, name="s2p")
                        nc.scalar.activation(junk[:], acc[:, cs], AF.Square,
                                             accum_out=s2p[:])
                        if s == 0:
                            nc.vector.tensor_copy(s2[:], s2p[:])
                        else:
                            nc.vector.tensor_add(s2[:], s2[:], s2p[:])
                    mu = stp.tile([128, 1], dt.float32, name="mu")
                    nc.vector.tensor_scalar_mul(mu[:], s1[:], 1.0 / H)
                    mu2 = stp.tile([128, 1], dt.float32, name="mu2")
                    nc.vector.tensor_mul(mu2[:], mu[:], mu[:])
                    var = stp.tile([128, 1], dt.float32, name="var")
                    nc.vector.tensor_scalar(var[:], s2[:], 1.0 / H, float(EPS),
                                            ALU.mult, ALU.add)
                    nc.vector.tensor_sub(var[:], var[:], mu2[:])
                    sd = stp.tile([128, 1], dt.float32, name="sd")
                    nc.scalar.activation(sd[:], var[:], AF.Sqrt)
                    rstd = stp.tile([128, 1], dt.float32, name="rstd")
                    nc.vector.reciprocal(rstd[:], sd[:])
                    nmr = stp.tile([128, 1], dt.float32, name="nmr")
                    nc.vector.tensor_mul(nmr[:], mu[:], rstd[:])
                    nc.vector.tensor_scalar_mul(nmr[:], nmr[:], -1.0)

                    for s in range(NS):
                        cs = slice(s * SW, (s + 1) * SW)
                        lnp = inp.tile([128, SW], dt.float32, name="rt")
                        nc.scalar.activation(lnp[:], acc[:, cs], AF.Identity,
                                             bias=nmr[:], scale=rstd[:])
                        for q in range(SW // 512):
                            pt = psum.tile([128, 512], dt.float32, name="ps")
                            for j in range(4):
                                nc.tensor.transpose(
                                    pt[:, j * 128:(j + 1) * 128],
                                    lnp[:, q * 512 + j * 128:
                                        q * 512 + (j + 1) * 128],
                                    ident[:])
                            for j in range(4):
                                k = (s * SW + q * 512) // 128 + j
                                nc.vector.tensor_scalar(
                                    lnT[:, k, t * 128:(t + 1) * 128],
                                    pt[:, j * 128:(j + 1) * 128],
                                    g_sb[:, k:k + 1], be_sb[:, k:k + 1],
                                    ALU.mult, ALU.add)

                # ---- Stage 2: per I-block GEMM1' -> gelu -> GEMM2' ----
                inter = interp.tile([128, IS, HALF], WDT, name="inter")
                for b in range(NB):
                    for g in range(IS // 4):        # groups of 4 i-slabs
                        pts = [psum.tile([128, HALF], dt.float32, name="ps")
                               for _ in range(4)]
                        for k in range(KS):
                            w1t = wtp.tile([128, 512], WDT, name="wt")
                            i0 = b * IBLK + g * 512
                            nc.sync.dma_start(
                                out=w1t[:],
                                in_=w1_d[k * 128:(k + 1) * 128, i0:i0 + 512])
                            for j in range(4):
                                nc.tensor.matmul(
                                    pts[j][:],
                                    w1t[:, j * 128:(j + 1) * 128],
                                    lnT[:, k, :],
                                    start=(k == 0), stop=(k == KS - 1))
                        for j in range(4):
                            slab = g * 4 + j
                            nc.scalar.activation(
                                inter[:, slab, :], pts[j][:],
                                AF.Gelu_apprx_tanh,
                                bias=b1_sb[:, b * IS + slab:
                                           b * IS + slab + 1])
                    last = (b == NB - 1)
                    for h in range(HC):
                        hs = slice(h * 512, (h + 1) * 512)
                        pBs = [psum.tile([128, 512], dt.float32, name="ps")
                               for _ in range(TT)]
                        for i in range(IS):
                            w2t = wtp.tile([128, 512], WDT, name="wt")
                            nc.sync.dma_start(
                                out=w2t[:],
                                in_=w2_d[b * IBLK + i * 128:
                                         b * IBLK + (i + 1) * 128, hs])
                            for t in range(TT):
                                nc.tensor.matmul(
                                    pBs[t][:],
                                    inter[:, i, t * 128:(t + 1) * 128],
                                    w2t[:],
                                    start=(i == 0),
                                    stop=(i == IS - 1) and not last)
                        if last:
                            for t in range(TT):
                                nc.tensor.matmul(pBs[t][:], ones[:],
                                                 b2_sb[:, hs],
                                                 start=False, stop=True)
                        for t in range(TT):
                            nc.vector.tensor_add(accs[t][:, hs],
                                                 accs[t][:, hs], pBs[t][:])

                for t in range(TT):
                    tr = slice(h0 + t * 128, h0 + (t + 1) * 128)
                    nc.sync.dma_start(out=o_d[tr, :], in_=accs[t][:])
    nc.compile()
    return nc


_NC_CACHE = None
_last_maps = None


def _get_nc():
    global _NC_CACHE
    if _NC_CACHE is None:
        _NC_CACHE = _build_nc(HIDDEN, INTER, T, IBLK)
    return _NC_CACHE


def _build_maps(input, residual, bias, attn_nw, attn_nb,
                inter_w, inter_b, output_w, output_b):
    H, I = HIDDEN, INTER
    KS = H // 128
    x = np.ascontiguousarray(np.asarray(input, np.float32).reshape(TOK, H))
    r2 = np.asarray(residual, np.float32).reshape(TOK, H) + \
        np.asarray(bias, np.float32)[None, :]
    gamma_t = np.ascontiguousarray(
        np.asarray(attn_nw, np.float32).reshape(KS, 128).T)
    beta_t = np.ascontiguousarray(
        np.asarray(attn_nb, np.float32).reshape(KS, 128).T)
    b1_t = np.ascontiguousarray(
        np.asarray(inter_b, np.float32).reshape(I // 128, 128).T)
    b2 = np.ascontiguousarray(np.asarray(output_b, WNP)[None, :])
    w1 = np.ascontiguousarray(np.asarray(inter_w, WNP))
    w2 = np.ascontiguousarray(np.asarray(output_w, WNP))
    ident = np.eye(128, dtype=np.float32)
    ones = np.ones((1, 128), WNP)

    maps = []
    for c in range(N_CORES):
        sl = slice(c * T, (c + 1) * T)
        maps.append({
            'x': x[sl], 'r': np.ascontiguousarray(r2[sl]),
            'gamma_t': gamma_t, 'beta_t': beta_t,
            'w1': w1, 'b1_t': b1_t, 'w2': w2, 'b2': b2,
            'ones': ones, 'ident': ident,
        })
    return maps


def kernel(input, residual, residual_norm, bias, attn_nw, attn_nb,
           inter_w, inter_b, output_w, output_b, **kwargs):
    global _last_maps
    nc = _get_nc()
    maps = _build_maps(input, residual, bias, attn_nw, attn_nb,
                       inter_w, inter_b, output_w, output_b)
    _last_maps = maps
    res = run_bass_kernel_spmd(nc, maps, list(range(N_CORES)))
    out = np.concatenate([res.results[c]['out'] for c in range(N_CORES)], 0)
    return out.reshape(B, S, HIDDEN).astype(np.float32)


# revision 4
# speedup vs baseline: 15.9231x; 1.0582x over previous
"""DeepSpeedMLP (pre-LN fp32 path) on 8 Trainium2 NeuronCores.

Sharding: data-parallel over tokens (8192 tokens -> 1024/core).  Measured
HW: 527 GB/s HBM per core (4.2 TB/s aggregate), 72.6 TF/s fp32r/bf16 per
core, so with bf16 weights (268 MB/core, streamed once per token-half)
weight DMA hides entirely under the ~3.8 ms of matmul.

Dataflow (the key change vs the naive version): GEMM1 is computed
TRANSPOSED -- psum[i,t] = sum_h w1[h,i] * lnT[h,t] -- so its output is
already in the [i-partition, t-free] layout that GEMM2 needs as lhsT.
Gelu+bias evicts PSUM straight into the SBUF `inter` buffer; no
mid-pipeline PE transposes, no staging round-trips.  GEMM2 partials are
accumulated into an SBUF-resident f32 accumulator that stage 1 seeded
with the residual (out = gemm2 + res + output_b), so the DRAM output is
write-once and the kernel never reads its own output buffer.

Per-core pipeline, per token-half (512 tokens; 2 halves so lnT + inter +
accumulator all fit in SBUF):
  stage 1: res = x + (residual+bias); LN stats; normalize; 128x128 PE
           transposes -> lnT[h-slab, t] bf16 (gamma/beta fused into the
           PSUM->SBUF eviction).  res tiles stay resident as the output
           accumulator.
  stage 2: per 2048-wide I-block:
           GEMM1' psum[i,t] += w1[h-slab, i-slab].T @ lnT[h-slab, t]
           evict: gelu(psum + b1) -> inter[i-slab, t] bf16
           GEMM2' psum[t,h] += inter[i-slab, t].T @ w2[i-slab, h]
                  (+ ones.T @ b2 folded into the last block)
           evict: acc[t,h] += psum (DVE)
  final: DMA acc -> out.
"""
import sys
if '/opt/trn_rl_repo' not in sys.path:
    sys.path.insert(0, '/opt/trn_rl_repo')

import numpy as np
import ml_dtypes
import concourse.bass as bass
import concourse.mybir as mybir
import concourse.tile as tile
from concourse import bacc
from concourse.bass_utils import run_bass_kernel_spmd

dt = mybir.dt
AF = mybir.ActivationFunctionType
ALU = mybir.AluOpType

N_CORES = 8
B, S, HIDDEN, INTER = 4, 2048, 4096, 16384
TOK = B * S
T = TOK // N_CORES       # tokens per core
HALF = T // 2            # tokens per half-pass
IBLK = 2048              # I-block width
EPS = 1e-5
WDT = dt.bfloat16        # matmul operand dtype (weights + activations)
WNP = ml_dtypes.bfloat16


def _build_nc(H, I, T, IBLK):
    HALF = T // 2        # tokens per half-pass
    KS = H // 128        # h-slabs (GEMM1' contraction steps)
    NB = I // IBLK       # I blocks
    IS = IBLK // 128     # i-slabs per block
    HC = H // 512        # 512-wide h-chunks (GEMM2' psum N)
    TT = HALF // 128     # token tiles per half
    SW = min(1024, H)    # stage-1 strip width
    NS = H // SW

    nc = bacc.Bacc(None, target_bir_lowering=False)
    P = nc.declare_dram_parameter
    x_d = P("x", [T, H], dt.float32, isOutput=False)
    r_d = P("r", [T, H], dt.float32, isOutput=False)
    g_d = P("gamma_t", [128, KS], dt.float32, isOutput=False)
    be_d = P("beta_t", [128, KS], dt.float32, isOutput=False)
    w1_d = P("w1", [H, I], WDT, isOutput=False)
    b1_d = P("b1_t", [128, I // 128], dt.float32, isOutput=False)
    w2_d = P("w2", [I, H], WDT, isOutput=False)
    b2_d = P("b2", [1, H], WDT, isOutput=False)
    ones_d = P("ones", [1, 128], WDT, isOutput=False)
    id_d = P("ident", [128, 128], dt.float32, isOutput=False)
    o_d = P("out", [T, H], dt.float32, isOutput=True)

    with tile.TileContext(nc) as tc:
        with (
            tc.tile_pool(name="const", bufs=1) as constp,
            tc.tile_pool(name="lnT", bufs=1) as lnTp,
            tc.tile_pool(name="acc", bufs=TT) as accp,
            tc.tile_pool(name="inter", bufs=1) as interp,
            tc.tile_pool(name="s1in", bufs=8) as inp,
            tc.tile_pool(name="s1st", bufs=20) as stp,
            tc.tile_pool(name="wt", bufs=8) as wtp,
            tc.tile_pool(name="psum", bufs=8, space="PSUM") as psum,
        ):
            ident = constp.tile([128, 128], dt.float32)
            nc.sync.dma_start(out=ident[:], in_=id_d[:])
            g_sb = constp.tile([128, KS], dt.float32)
            nc.sync.dma_start(out=g_sb[:], in_=g_d[:])
            be_sb = constp.tile([128, KS], dt.float32)
            nc.sync.dma_start(out=be_sb[:], in_=be_d[:])
            b1_sb = constp.tile([128, I // 128], dt.float32)
            nc.sync.dma_start(out=b1_sb[:], in_=b1_d[:])
            ones = constp.tile([1, 128], WDT)
            nc.sync.dma_start(out=ones[:], in_=ones_d[:])
            b2_sb = constp.tile([1, H], WDT)
            nc.sync.dma_start(out=b2_sb[:], in_=b2_d[:])

            for half in range(2):
                h0 = half * HALF
                lnT = lnTp.tile([128, KS, HALF], WDT, name="lnT")
                accs = []

                # ---- Stage 1: residual add + LN + transpose ----
                for t in range(TT):
                    tr = slice(h0 + t * 128, h0 + (t + 1) * 128)
                    acc = accp.tile([128, H], dt.float32, name="acc")
                    accs.append(acc)
                    for s in range(NS):
                        cs = slice(s * SW, (s + 1) * SW)
                        xs = inp.tile([128, SW], dt.float32, name="xt")
                        rs = inp.tile([128, SW], dt.float32, name="rt")
                        nc.sync.dma_start(out=xs[:], in_=x_d[tr, cs])
                        nc.sync.dma_start(out=rs[:], in_=r_d[tr, cs])
                        nc.vector.tensor_add(acc[:, cs], xs[:], rs[:])

                    s1 = stp.tile([128, 1], dt.float32, name="s1")
                    nc.vector.tensor_reduce(s1[:], acc[:], mybir.AxisListType.X,
                                            ALU.add)
                    s2 = stp.tile([128, 1], dt.float32, name="s2")
                    for s in range(NS):
                        cs = slice(s * SW, (s + 1) * SW)
                        junk = inp.tile([128, SW], dt.float32, name="xt")
                        s2p = stp.tile([128, 1], dt.float32, name="s2p")
                        nc.scalar.activation(junk[:], acc[:, cs], AF.Square,
                                             accum_out=s2p[:])
                        if s == 0:
                            nc.vector.tensor_copy(s2[:], s2p[:])
                        else:
                            nc.vector.tensor_add(s2[:], s2[:], s2p[:])
                    mu = stp.tile([128, 1], dt.float32, name="mu")
                    nc.vector.tensor_scalar_mul(mu[:], s1[:], 1.0 / H)
                    mu2 = stp.tile([128, 1], dt.float32, name="mu2")
                    nc.vector.tensor_mul(mu2[:], mu[:], mu[:])
                    var = stp.tile([128, 1], dt.float32, name="var")
                    nc.vector.tensor_scalar(var[:], s2[:], 1.0 / H, float(EPS),
                                            ALU.mult, ALU.add)
                    nc.vector.tensor_sub(var[:], var[:], mu2[:])
                    sd = stp.tile([128, 1], dt.float32, name="sd")
                    nc.scalar.activation(sd[:], var[:], AF.Sqrt)
                    rstd = stp.tile([128, 1], dt.float32, name="rstd")
                    nc.vector.reciprocal(rstd[:], sd[:])
                    nmr = stp.tile([128, 1], dt.float32, name="nmr")
                    nc.vector.tensor_mul(nmr[:], mu[:], rstd[:])
                    nc.vector.tensor_scalar_mul(nmr[:], nmr[:], -1.0)

                    for s in range(NS):
                        cs = slice(s * SW, (s + 1) * SW)
                        lnp = inp.tile([128, SW], dt.float32, name="rt")
                        nc.scalar.activation(lnp[:], acc[:, cs], AF.Identity,
                                             bias=nmr[:], scale=rstd[:])
                        for q in range(SW // 512):
                            pt = psum.tile([128, 512], dt.float32, name="ps")
                            for j in range(4):
                                nc.tensor.transpose(
                                    pt[:, j * 128:(j + 1) * 128],
                                    lnp[:, q * 512 + j * 128:
                                        q * 512 + (j + 1) * 128],
                                    ident[:])
                            for j in range(4):
                                k = (s * SW + q * 512) // 128 + j
                                nc.vector.tensor_scalar(
                                    lnT[:, k, t * 128:(t + 1) * 128],
                                    pt[:, j * 128:(j + 1) * 128],
                                    g_sb[:, k:k + 1], be_sb[:, k:k + 1],
                                    ALU.mult, ALU.add)

                # ---- Stage 2: per I-block GEMM1' -> gelu -> GEMM2' ----
                inter = interp.tile([128, IS, HALF], WDT, name="inter")
                for b in range(NB):
                    for g in range(IS // 4):        # groups of 4 i-slabs
                        pts = [psum.tile([128, HALF], dt.float32, name="ps")
                               for _ in range(4)]
                        for k in range(KS):
                            w1t = wtp.tile([128, 512], WDT, name="wt")
                            i0 = b * IBLK + g * 512
                            nc.sync.dma_start(
                                out=w1t[:],
                                in_=w1_d[k * 128:(k + 1) * 128, i0:i0 + 512])
                            for j in range(4):
                                nc.tensor.matmul(
                                    pts[j][:],
                                    w1t[:, j * 128:(j + 1) * 128],
                                    lnT[:, k, :],
                                    start=(k == 0), stop=(k == KS - 1))
                        for j in range(4):
                            slab = g * 4 + j
                            nc.scalar.activation(
                                inter[:, slab, :], pts[j][:],
                                AF.Gelu_apprx_tanh,
                                bias=b1_sb[:, b * IS + slab:
                                           b * IS + slab + 1])
                    last = (b == NB - 1)
                    for h in range(HC):
                        hs = slice(h * 512, (h + 1) * 512)
                        pBs = [psum.tile([128, 512], dt.float32, name="ps")
                               for _ in range(TT)]
                        for i in range(IS):
                            w2t = wtp.tile([128, 512], WDT, name="wt")
                            nc.sync.dma_start(
                                out=w2t[:],
                                in_=w2_d[b * IBLK + i * 128:
                                         b * IBLK + (i + 1) * 128, hs])
                            for t in range(TT):
                                nc.tensor.matmul(
                                    pBs[t][:],
                                    inter[:, i, t * 128:(t + 1) * 128],
                                    w2t[:],
                                    start=(i == 0),
                                    stop=(i == IS - 1) and not last)
                        if last:
                            for t in range(TT):
                                nc.tensor.matmul(pBs[t][:], ones[:],
                                                 b2_sb[:, hs],
                                                 start=False, stop=True)
                        for t in range(TT):
                            nc.vector.tensor_add(accs[t][:, hs],
                                                 accs[t][:, hs], pBs[t][:])

                for t in range(TT):
                    tr = slice(h0 + t * 128, h0 + (t + 1) * 128)
                    nc.sync.dma_start(out=o_d[tr, :], in_=accs[t][:])
    nc.compile()
    return nc


_NC_CACHE = None
_last_maps = None


def _get_nc():
    global _NC_CACHE
    if _NC_CACHE is None:
        _NC_CACHE = _build_nc(HIDDEN, INTER, T, IBLK)
    return _NC_CACHE


def _build_maps(input, residual, bias, attn_nw, attn_nb,
                inter_w, inter_b, output_w, output_b):
    H, I = HIDDEN, INTER
    KS = H // 128
    x = np.ascontiguousarray(np.asarray(input, np.float32).reshape(TOK, H))
    r2 = np.asarray(residual, np.float32).reshape(TOK, H) + \
        np.asarray(bias, np.float32)[None, :]
    gamma_t = np.ascontiguousarray(
        np.asarray(attn_nw, np.float32).reshape(KS, 128).T)
    beta_t = np.ascontiguousarray(
        np.asarray(attn_nb, np.float32).reshape(KS, 128).T)
    b1_t = np.ascontiguousarray(
        np.asarray(inter_b, np.float32).reshape(I // 128, 128).T)
    b2 = np.ascontiguousarray(np.asarray(output_b, WNP)[None, :])
    w1 = np.ascontiguousarray(np.asarray(inter_w, WNP))
    w2 = np.ascontiguousarray(np.asarray(output_w, WNP))
    ident = np.eye(128, dtype=np.float32)
    ones = np.ones((1, 128), WNP)

    maps = []
    for c in range(N_CORES):
        sl = slice(c * T, (c + 1) * T)
        maps.append({
            'x': x[sl], 'r': np.ascontiguousarray(r2[sl]),
            'gamma_t': gamma_t, 'beta_t': beta_t,
            'w1': w1, 'b1_t': b1_t, 'w2': w2, 'b2': b2,
            'ones': ones, 'ident': ident,
        })
    return maps


def kernel(input, residual, residual_norm, bias, attn_nw, attn_nb,
           inter_w, inter_b, output_w, output_b, **kwargs):
    global _last_maps
    nc = _get_nc()
    maps = _build_maps(input, residual, bias, attn_nw, attn_nb,
                       inter_w, inter_b, output_w, output_b)
    _last_maps = maps
    res = run_bass_kernel_spmd(nc, maps, list(range(N_CORES)))
    out = np.concatenate([res.results[c]['out'] for c in range(N_CORES)], 0)
    return out.reshape(B, S, HIDDEN).astype(np.float32)
